# revision 3
# baseline (speedup 1.0000x reference)
"""BDH forward (nn_BDH_4406636445721) on 8 TRN2 NeuronCores via Bass/Tile.

Sharding: core c -> head h=c//2, n-half=c%2 (4096 of 8192 latent rows).
Within a core's n-slice, n is permuted to [evens; odds] so RoPE pairs
(2i, 2i+1) become rows i (E block) and 2048+i (O block); encoder /
encoder_v columns and decoder rows get the same permutation, which is
transparent to every contraction over n.

Per layer (all tensors stored transposed [n, t] so PE contracts over
partitions):
  x_spT = relu(enc^T @ xT)                 PE, spilled to HBM for reuse
  qrT   = rope(x_spT)                      DVE+GPSIMD, cos/sin streamed
  scoresT[s,t] = sum_n qrT qrT             PE, upper-tri blocks only
  ykv_partial = scoresT(masked)^T-contract PE  -> pair AllReduce (ykv is
                                               linear in scores!)
  ykvn = LN(ykv) -> ykvT                   DVE + PE transpose
  y_spT = relu(encv^T @ ykvT)              PE
  xyT   = x_spT * y_spT                    DVE
  ymlp_partial = xyT^T @ dec               PE  -> 8-core AllReduce
  x = LN(x + LN(ymlp))                     DVE/ACT, replicated
Final: logits = x @ lm_head in fp32.
"""
import math
from contextlib import ExitStack

import numpy as np
import ml_dtypes

import concourse.bass as bass
import concourse.tile as tile
from concourse import bacc, mybir
from concourse.bass_utils import run_bass_kernel_spmd
from concourse.masks import make_identity

P = 128
T, D, NH, N, NL, VOCAB = 1024, 256, 4, 8192, 6, 256
NSH = N // 2            # n rows per core
F = NSH // 2            # rope pairs per core
NT = NSH // P           # 32 n-tiles
FT = F // P             # 16 pair-tiles (E tiles)
KT = T // P             # 8 t-tiles
DT = D // P             # 2 d-subtiles
THETA = 2.0 ** 16
TWO_PI = 2.0 * math.pi
EPS = 1e-5

BF = mybir.dt.bfloat16
FP = mybir.dt.float32
bf16 = ml_dtypes.bfloat16

_CACHE = {}


# --------------------------------------------------------------------------
# device program
# --------------------------------------------------------------------------

def _build_nc():
    nc = bacc.Bacc("TRN2", target_bir_lowering=False, debug=False, num_devices=8)

    d_x0 = nc.dram_tensor("x0", [T, D], FP, kind="ExternalInput")
    d_cos = nc.dram_tensor("cosT", [F, T], BF, kind="ExternalInput")
    d_sin = nc.dram_tensor("sinT", [F, T], BF, kind="ExternalInput")
    d_enc = nc.dram_tensor("enc", [P, DT, NSH], BF, kind="ExternalInput")
    d_encv = nc.dram_tensor("encv", [P, DT, NSH], BF, kind="ExternalInput")
    d_dec = nc.dram_tensor("dec", [P, NT, D], BF, kind="ExternalInput")
    d_mask = nc.dram_tensor("maskT", [P, P], BF, kind="ExternalInput")
    d_lmh = nc.dram_tensor("lmh", [P, DT, VOCAB], FP, kind="ExternalInput")
    d_out = nc.dram_tensor("logits", [T, VOCAB], FP, kind="ExternalOutput")

    Relu = mybir.ActivationFunctionType.Relu
    Sqrt = mybir.ActivationFunctionType.Sqrt
    MUL = mybir.AluOpType.mult
    SUB = mybir.AluOpType.subtract
    ADD = mybir.AluOpType.add

    with ExitStack() as ctx:
        tc = ctx.enter_context(tile.TileContext(nc))
        singles = ctx.enter_context(tc.tile_pool(name="singles", bufs=1))
        big = ctx.enter_context(tc.tile_pool(name="big", bufs=NT))        # qr/xy slots
        big2 = ctx.enter_context(tc.tile_pool(name="big2", bufs=1))       # scoresT
        xsp_pool = ctx.enter_context(tc.tile_pool(name="xsp", bufs=6))
        tmp_pool = ctx.enter_context(tc.tile_pool(name="tmp", bufs=2))
        trig_pool = ctx.enter_context(tc.tile_pool(name="trig", bufs=2))
        w_pool = ctx.enter_context(tc.tile_pool(name="w", bufs=4))
        dec_pool = ctx.enter_context(tc.tile_pool(name="decw", bufs=4))
        y_pool = ctx.enter_context(tc.tile_pool(name="y", bufs=3))
        rel_pool = ctx.enter_context(tc.tile_pool(name="rel", bufs=4))
        cp_pool = ctx.enter_context(tc.tile_pool(name="cp", bufs=3))
        st_pool = ctx.enter_context(tc.tile_pool(name="st", bufs=8))
        kv_pool = ctx.enter_context(tc.tile_pool(name="kv", bufs=1))
        psA = ctx.enter_context(tc.tile_pool(name="psA", bufs=4, space="PSUM"))
        psB = ctx.enter_context(tc.tile_pool(name="psB", bufs=2, space="PSUM"))
        psT = ctx.enter_context(tc.tile_pool(name="psT", bufs=2, space="PSUM"))
        dram = ctx.enter_context(tc.tile_pool(name="dram", bufs=2, space="DRAM"))
        drsh = ctx.enter_context(tc.tile_pool(name="drsh", bufs=2, space="DRAM"))

        # ---- persistent state ----
        s_mask = singles.tile([P, P], BF)
        nc.sync.dma_start(s_mask[:], d_mask[:])
        s_lmh = singles.tile([P, DT, VOCAB], FP)
        nc.sync.dma_start(s_lmh[:], d_lmh[:])
        ident_bf = singles.tile([P, P], BF)
        make_identity(nc, ident_bf[:])
        ident_f = singles.tile([P, P], FP)
        make_identity(nc, ident_f[:])
        s_eps = singles.tile([P, 1], FP)
        nc.vector.memset(s_eps[:], EPS)

        s_x = singles.tile([P, KT, D], FP)       # x natural fp32 (t_in, t_out, d)
        s_xbf = singles.tile([P, KT, D], BF)     # x natural bf16
        s_xT = singles.tile([P, DT, T], BF)      # xT bf16 (d_in, d_out, t)

        nc.sync.dma_start(s_x[:], d_x0.ap().rearrange("(o p) d -> p o d", p=P))
        nc.vector.tensor_copy(s_xbf[:], s_x[:])

        def transpose_x_to(dst_bf):
            # dst[d_in, dc, t] = x[t, d] transposed, from s_xbf
            for k in range(KT):
                for dc in range(DT):
                    tp = psT.tile([P, P], BF, tag="trbf")
                    nc.tensor.transpose(
                        tp[:], s_xbf[:, k, dc * P:(dc + 1) * P], ident_bf[:])
                    nc.any.tensor_copy(
                        dst_bf[:, dc, k * P:(k + 1) * P], tp[:])

        transpose_x_to(s_xT)

        # ================= layers =================
        for layer in range(NL):
            # ---------- phase A: x_spT + rope ----------
            spill = dram.tile([NSH, T], BF, tag="spill")
            qr = [None] * NT
            xsp_sb = {}
            for pi in range(FT):
                for idx in (pi, FT + pi):            # E tile then O tile
                    wt = w_pool.tile([P, DT, P], BF, tag="enc")
                    nc.sync.dma_start(wt[:], d_enc[:, :, idx * P:(idx + 1) * P])
                    sb = xsp_pool.tile([P, T], BF, tag="xsp")
                    for th in range(2):
                        ps = psA.tile([P, 512], FP, tag="mm")
                        for dc in range(DT):
                            nc.tensor.matmul(
                                ps[:], wt[:, dc, :],
                                s_xT[:, dc, th * 512:(th + 1) * 512],
                                start=(dc == 0), stop=(dc == DT - 1))
                        nc.scalar.activation(sb[:, th * 512:(th + 1) * 512], ps[:], Relu)
                    nc.sync.dma_start(spill[idx * P:(idx + 1) * P, :], sb[:])
                    xsp_sb[idx] = sb
                ce = trig_pool.tile([P, T], BF, tag="cos")
                nc.sync.dma_start(ce[:], d_cos[pi * P:(pi + 1) * P, :])
                se = trig_pool.tile([P, T], BF, tag="sin")
                nc.sync.dma_start(se[:], d_sin[pi * P:(pi + 1) * P, :])
                E, O = xsp_sb.pop(pi), xsp_sb.pop(FT + pi)
                qe = big.tile([P, T], BF, tag="big")
                qo = big.tile([P, T], BF, tag="big")
                t2 = tmp_pool.tile([P, T], BF, tag="t2")
                t4 = tmp_pool.tile([P, T], BF, tag="t4")
                nc.vector.tensor_tensor(qe[:], E[:], ce[:], MUL)     # E*cos
                nc.vector.tensor_tensor(t2[:], O[:], se[:], MUL)     # O*sin
                nc.vector.tensor_tensor(qe[:], qe[:], t2[:], SUB)
                nc.vector.tensor_tensor(qo[:], O[:], ce[:], MUL)     # O*cos
                nc.gpsimd.tensor_tensor(t4[:], E[:], se[:], MUL)     # E*sin
                nc.gpsimd.tensor_tensor(qo[:], qo[:], t4[:], ADD)
                qr[pi], qr[FT + pi] = qe, qo

            # ---------- phase B: scoresT (upper-tri blocks) ----------
            s_scT = big2.tile([P, KT, T], BF, tag="scT")
            for i_s in range(KT):
                for j in range(i_s // 4, 2):
                    ps = psA.tile([P, 512], FP, tag="mm")
                    for n in range(NT):
                        nc.tensor.matmul(
                            ps[:], qr[n][:, i_s * P:(i_s + 1) * P],
                            qr[n][:, j * 512:(j + 1) * 512],
                            start=(n == 0), stop=(n == NT - 1))
                    dst = s_scT[:, i_s, j * 512:(j + 1) * 512]
                    c0 = i_s * P - j * 512
                    if 0 <= c0 < 512:
                        # diagonal 128-col sub-block -> strict mask; cols left
                        # of it are never read (t<s region), copy the rest
                        nc.vector.tensor_tensor(
                            dst[:, c0:c0 + P], ps[:, c0:c0 + P], s_mask[:], MUL)
                        if c0 + P < 512:
                            nc.any.tensor_copy(dst[:, c0 + P:], ps[:, c0 + P:])
                    else:
                        nc.any.tensor_copy(dst[:], ps[:])

            # ---------- phase C: ykv partial + pair AllReduce ----------
            ykv_in = dram.tile([T, D], FP, tag="ykv_in")
            ykv_out = dram.tile([T, D], FP, tag="ykv_out")
            for k in range(KT):
                ps = psB.tile([P, D], FP, tag="pb")
                for i in range(k + 1):
                    nc.tensor.matmul(
                        ps[:], s_scT[:, i, k * P:(k + 1) * P], s_xbf[:, i, :],
                        start=(i == 0), stop=(i == k))
                cp = cp_pool.tile([P, D], FP, tag="cp")
                nc.any.tensor_copy(cp[:], ps[:])
                nc.sync.dma_start(ykv_in[k * P:(k + 1) * P, :], cp[:])
            nc.gpsimd.collective_compute(
                "AllReduce", ADD,
                replica_groups=[[0, 1], [2, 3], [4, 5], [6, 7]],
                ins=[ykv_in.opt()], outs=[ykv_out.opt()])

            # ---------- phase D: LN(ykv) -> ykvT ----------
            s_ykv = kv_pool.tile([P, KT, D], FP, tag="ykv")
            nc.sync.dma_start(s_ykv[:], ykv_out.rearrange("(o p) d -> p o d", p=P))
            s_ykvn = kv_pool.tile([P, KT, D], BF, tag="ykvn")
            s_ykvT = kv_pool.tile([P, DT, T], BF, tag="ykvT")
            for k in range(KT):
                stats = st_pool.tile([P, 6], FP, tag="bn")
                nc.vector.bn_stats(stats[:], s_ykv[:, k, :])
                mv = st_pool.tile([P, 2], FP, tag="mv")
                nc.vector.bn_aggr(mv[:], stats[:])
                rstd = st_pool.tile([P, 1], FP, tag="rstd")
                nc.scalar.activation(rstd[:], mv[:, 1:2], Sqrt, bias=s_eps[:])
                nc.vector.reciprocal(rstd[:], rstd[:])
                nc.vector.tensor_scalar(
                    s_ykvn[:, k, :], s_ykv[:, k, :],
                    mv[:, 0:1], rstd[:], SUB, MUL)
                for dc in range(DT):
                    tp = psT.tile([P, P], BF, tag="trbf")
                    nc.tensor.transpose(
                        tp[:], s_ykvn[:, k, dc * P:(dc + 1) * P], ident_bf[:])
                    nc.any.tensor_copy(s_ykvT[:, dc, k * P:(k + 1) * P], tp[:])

            # ---------- phase Y: y_spT, xyT ----------
            xy = [None] * NT
            for i in range(NT):
                wt = w_pool.tile([P, DT, P], BF, tag="encv")
                nc.sync.dma_start(wt[:], d_encv[:, :, i * P:(i + 1) * P])
                xr = rel_pool.tile([P, T], BF, tag="rel")
                nc.sync.dma_start(xr[:], spill[i * P:(i + 1) * P, :])
                xy_i = big.tile([P, T], BF, tag="big")
                for th in range(2):
                    ps = psA.tile([P, 512], FP, tag="mm")
                    for dc in range(DT):
                        nc.tensor.matmul(
                            ps[:], wt[:, dc, :],
                            s_ykvT[:, dc, th * 512:(th + 1) * 512],
                            start=(dc == 0), stop=(dc == DT - 1))
                    yt = y_pool.tile([P, 512], BF, tag="yt")
                    nc.scalar.activation(yt[:], ps[:], Relu)
                    nc.vector.tensor_tensor(
                        xy_i[:, th * 512:(th + 1) * 512],
                        xr[:, th * 512:(th + 1) * 512], yt[:], MUL)
                xy[i] = xy_i

            # ---------- phase M: ymlp partial + 8-core AllReduce ----------
            ymlp_in = dram.tile([T, D], FP, tag="ymlp_in")
            ymlp_out = drsh.tile([T, D], FP, tag="ymlp_out", addr_space="Shared")
            for kg in range(2):
                pss = [psA.tile([P, 512], FP, tag="mm", name=f"pss_{layer}_{kg}_{k4}")
                       for k4 in range(4)]
                for i in range(NT):
                    dw = dec_pool.tile([P, D], BF, tag="dec")
                    nc.sync.dma_start(dw[:], d_dec[:, i, :])
                    for k4 in range(4):
                        k = kg * 4 + k4
                        nc.tensor.matmul(
                            pss[k4][:, :D], xy[i][:, k * P:(k + 1) * P], dw[:],
                            start=(i == 0), stop=(i == NT - 1))
                for k4 in range(4):
                    k = kg * 4 + k4
                    cp = cp_pool.tile([P, D], FP, tag="cp")
                    nc.any.tensor_copy(cp[:], pss[k4][:, :D])
                    nc.sync.dma_start(ymlp_in[k * P:(k + 1) * P, :], cp[:])
            nc.gpsimd.collective_compute(
                "AllReduce", ADD,
                replica_groups=[[0, 1, 2, 3, 4, 5, 6, 7]],
                ins=[ymlp_in.opt()], outs=[ymlp_out.opt()])

            # ---------- phase X: x = LN(x + LN(ymlp)) ----------
            s_ym = kv_pool.tile([P, KT, D], FP, tag="ym")
            nc.sync.dma_start(s_ym[:], ymlp_out.rearrange("(o p) d -> p o d", p=P))
            for k in range(KT):
                stats = st_pool.tile([P, 6], FP, tag="bn")
                nc.vector.bn_stats(stats[:], s_ym[:, k, :])
                mv = st_pool.tile([P, 2], FP, tag="mv")
                nc.vector.bn_aggr(mv[:], stats[:])
                rstd = st_pool.tile([P, 1], FP, tag="rstd")
                nc.scalar.activation(rstd[:], mv[:, 1:2], Sqrt, bias=s_eps[:])
                nc.vector.reciprocal(rstd[:], rstd[:])
                u = cp_pool.tile([P, D], FP, tag="u")
                nc.vector.tensor_scalar(
                    u[:], s_ym[:, k, :], mv[:, 0:1], rstd[:], SUB, MUL)
                nc.vector.tensor_tensor(u[:], u[:], s_x[:, k, :], ADD)
                stats2 = st_pool.tile([P, 6], FP, tag="bn")
                nc.vector.bn_stats(stats2[:], u[:])
                mv2 = st_pool.tile([P, 2], FP, tag="mv")
                nc.vector.bn_aggr(mv2[:], stats2[:])
                rstd2 = st_pool.tile([P, 1], FP, tag="rstd")
                nc.scalar.activation(rstd2[:], mv2[:, 1:2], Sqrt, bias=s_eps[:])
                nc.vector.reciprocal(rstd2[:], rstd2[:])
                nc.vector.tensor_scalar(
                    s_x[:, k, :], u[:], mv2[:, 0:1], rstd2[:], SUB, MUL)
                nc.vector.tensor_copy(s_xbf[:, k, :], s_x[:, k, :])
            transpose_x_to(s_xT)

        # ================= final head (fp32) =================
        s_xTf = big2.tile([P, DT, T], FP, tag="scT")
        for k in range(KT):
            for dc in range(DT):
                tp = psB.tile([P, D], FP, tag="pb")
                nc.tensor.transpose(
                    tp[:, :P], s_x[:, k, dc * P:(dc + 1) * P], ident_f[:])
                nc.any.tensor_copy(s_xTf[:, dc, k * P:(k + 1) * P], tp[:, :P])
        for k in range(KT):
            ps = psB.tile([P, D], FP, tag="pb")
            for dc in range(DT):
                nc.tensor.matmul(
                    ps[:], s_xTf[:, dc, k * P:(k + 1) * P], s_lmh[:, dc, :],
                    start=(dc == 0), stop=(dc == DT - 1))
            cp = cp_pool.tile([P, VOCAB], FP, tag="cpo")
            nc.any.tensor_copy(cp[:], ps[:])
            nc.sync.dma_start(d_out[k * P:(k + 1) * P, :], cp[:])

    nc.compile()
    return nc


# --------------------------------------------------------------------------
# host side
# --------------------------------------------------------------------------

def _ln_np(x, eps=EPS):
    m = x.mean(-1, keepdims=True)
    v = ((x - m) ** 2).mean(-1, keepdims=True)
    return (x - m) / np.sqrt(v + eps)


def _prep_in_maps(inputs):
    idx = np.asarray(inputs["idx"]).reshape(-1).astype(np.int64)
    embed_w = np.asarray(inputs["embed_w"], dtype=np.float32)
    encoder = np.asarray(inputs["encoder"], dtype=np.float32)
    encoder_v = np.asarray(inputs["encoder_v"], dtype=np.float32)
    decoder = np.asarray(inputs["decoder"], dtype=np.float32).reshape(NH, N, D)
    lm_head = np.asarray(inputs["lm_head"], dtype=np.float32)

    x0 = _ln_np(embed_w[idx]).astype(np.float32)

    # freqs/phases mirrored from the reference in f32
    t_ = np.arange(N, dtype=np.float32)
    q = np.floor(t_ / np.float32(2.0)) * np.float32(2.0)
    freqs = (np.float32(1.0) / (np.float32(THETA) ** (q / np.float32(N)))
             / np.float32(TWO_PI))
    tt = np.arange(T, dtype=np.float32)
    phases = tt[:, None] * freqs[None, :]
    ph = np.mod(phases, np.float32(1.0)).astype(np.float32) * np.float32(TWO_PI)
    cos_full = np.cos(ph).astype(np.float32)
    sin_full = np.sin(ph).astype(np.float32)

    maskT = (np.arange(P)[:, None] < np.arange(P)[None, :]).astype(bf16)
    lmh = np.ascontiguousarray(
        lm_head.reshape(DT, P, VOCAB).transpose(1, 0, 2)).astype(np.float32)

    in_maps = []
    for c in range(8):
        h, half = c // 2, c % 2
        base = half * NSH
        perm = np.concatenate(
            [base + 2 * np.arange(F), base + 2 * np.arange(F) + 1])
        enc_c = encoder[h][:, perm]                     # [256, 4096]
        encv_c = encoder_v[h][:, perm]
        dec_c = decoder[h][perm, :]                     # [4096, 256]
        pcols = 2 * (half * F + np.arange(F))
        in_maps.append({
            "x0": x0,
            "cosT": np.ascontiguousarray(cos_full[:, pcols].T).astype(bf16),
            "sinT": np.ascontiguousarray(sin_full[:, pcols].T).astype(bf16),
            "enc": np.ascontiguousarray(
                enc_c.reshape(DT, P, NSH).transpose(1, 0, 2)).astype(bf16),
            "encv": np.ascontiguousarray(
                encv_c.reshape(DT, P, NSH).transpose(1, 0, 2)).astype(bf16),
            "dec": np.ascontiguousarray(
                dec_c.reshape(NT, P, D).transpose(1, 0, 2)).astype(bf16),
            "maskT": maskT,
            "lmh": lmh,
        })
    return in_maps


def kernel(**inputs) -> np.ndarray:
    in_maps = _prep_in_maps(inputs)
    if "nc" not in _CACHE:
        _CACHE["nc"] = _build_nc()
    res = run_bass_kernel_spmd(_CACHE["nc"], in_maps, core_ids=list(range(8)))
    logits = np.asarray(res.results[0]["logits"], dtype=np.float32)
    return logits.reshape(1, T, VOCAB)


# revision 12
# speedup vs baseline: 41.7065x; 41.7065x over previous
"""BDH forward (nn_BDH_4406636445721) on 8 TRN2 NeuronCores via Bass/Tile.

Sharding: core c -> head h=c//2, n-half=c%2 (4096 of 8192 latent rows).
Within a core's n-slice, n is permuted to [evens; odds] so RoPE pairs
(2i, 2i+1) become rows i (E block) and 2048+i (O block); encoder /
encoder_v columns and decoder rows get the same permutation, which is
transparent to every contraction over n.

The layer is software-pipelined over t-HALVES (t is never a contraction
dim, and causality means t-half0 only ever needs s<512): every AllReduce
(pair-AR of ykv partials, 8-core AR of ymlpT partials, both bf16)
overlaps compute of the other half / next layer.

Per layer and t-half (big tensors stored transposed [n, t]):
  x_spT = relu(enc^T @ xT)                  PE, spilled to HBM
  qrT   = rope(x_spT)                       DVE+GPSIMD
  scoresT[s,t] = sum_n qrT qrT              PE, upper-tri blocks only
  ykv_partial = masked scoresT contraction  PE -> pair AllReduce (ykv is
                                            linear in scores)
  ykvn = LN(ykv) -> ykvT                    DVE + PE transpose
  y_spT = relu(encv^T @ ykvT)               PE
  xyT   = x_spT * y_spT                     DVE
  ymlpT_partial = dec-stationary contract   PE -> 8-core AllReduce
  x = LN(x + LN(ymlp))                      DVE/ACT, replicated
Final: logits = x @ lm_head in fp32.
"""
import math
from contextlib import ExitStack

import numpy as np
import ml_dtypes

import concourse.bass as bass
import concourse.tile as tile
from concourse import bacc, mybir
from concourse.bass_utils import run_bass_kernel_spmd
from concourse.masks import make_identity

P = 128
T, D, NH, N, NL, VOCAB = 1024, 256, 4, 8192, 6, 256
NSH = N // 2            # n rows per core
F = NSH // 2            # rope pairs per core
NT = NSH // P           # 32 n-tiles
FT = F // P             # 16 pair-tiles (E tiles)
KT = T // P             # 8 t-tiles
DT = D // P             # 2 d-subtiles
H = 512                 # t-half width
KH = KT // 2            # t-tiles per half
THETA = 2.0 ** 16
TWO_PI = 2.0 * math.pi
EPS = 1e-5

BF = mybir.dt.bfloat16
FP = mybir.dt.float32
bf16 = ml_dtypes.bfloat16

_CACHE = {}


def _build_nc(reps=1):
    nc = bacc.Bacc("TRN2", target_bir_lowering=False, debug=False, num_devices=8)

    d_x0 = nc.dram_tensor("x0", [T, D], FP, kind="ExternalInput")
    d_cos = nc.dram_tensor("cosT", [F, T], BF, kind="ExternalInput")
    d_sin = nc.dram_tensor("sinT", [F, T], BF, kind="ExternalInput")
    d_enc = nc.dram_tensor("enc", [P, DT, NSH], BF, kind="ExternalInput")
    d_encv = nc.dram_tensor("encv", [P, DT, NSH], BF, kind="ExternalInput")
    d_dec = nc.dram_tensor("dec", [P, NT, D], BF, kind="ExternalInput")
    d_mask = nc.dram_tensor("maskT", [P, P], BF, kind="ExternalInput")
    d_lmh = nc.dram_tensor("lmh", [P, DT, VOCAB], FP, kind="ExternalInput")
    d_out = nc.dram_tensor("logits", [T, VOCAB], FP, kind="ExternalOutput")

    Relu = mybir.ActivationFunctionType.Relu
    Sqrt = mybir.ActivationFunctionType.Sqrt
    MUL = mybir.AluOpType.mult
    SUB = mybir.AluOpType.subtract
    ADD = mybir.AluOpType.add

    with ExitStack() as ctx:
        tc = ctx.enter_context(tile.TileContext(nc))
        singles = ctx.enter_context(tc.tile_pool(name="singles", bufs=1))
        big = ctx.enter_context(tc.tile_pool(name="big", bufs=2 * NT))
        big2 = ctx.enter_context(tc.tile_pool(name="big2", bufs=2))
        stage = ctx.enter_context(tc.tile_pool(name="stage", bufs=4))
        tmp_pool = ctx.enter_context(tc.tile_pool(name="tmp", bufs=2))
        trig_pool = ctx.enter_context(tc.tile_pool(name="trig", bufs=4))
        w_pool = ctx.enter_context(tc.tile_pool(name="w", bufs=4))
        y_pool = ctx.enter_context(tc.tile_pool(name="y", bufs=3))
        cp_pool = ctx.enter_context(tc.tile_pool(name="cp", bufs=3))
        ar_pool = ctx.enter_context(tc.tile_pool(name="arst", bufs=3))
        st_pool = ctx.enter_context(tc.tile_pool(name="st", bufs=3))
        u_pool = ctx.enter_context(tc.tile_pool(name="u", bufs=KH))
        kv_pool = ctx.enter_context(tc.tile_pool(name="kv", bufs=2))
        psA = ctx.enter_context(tc.tile_pool(name="psA", bufs=4, space="PSUM"))
        psB = ctx.enter_context(tc.tile_pool(name="psB", bufs=2, space="PSUM"))
        psT = ctx.enter_context(tc.tile_pool(name="psT", bufs=2, space="PSUM"))
        dram = ctx.enter_context(tc.tile_pool(name="dram", bufs=2, space="DRAM"))
        drsh = ctx.enter_context(tc.tile_pool(name="drsh", bufs=4, space="DRAM"))

        # ---- persistent state ----
        s_mask = singles.tile([P, P], BF)
        nc.sync.dma_start(s_mask[:], d_mask[:])
        s_lmh = singles.tile([P, DT, VOCAB], FP)
        nc.sync.dma_start(s_lmh[:], d_lmh[:])
        ident_bf = singles.tile([P, P], BF)
        make_identity(nc, ident_bf[:])
        ident_f = singles.tile([P, P], FP)
        make_identity(nc, ident_f[:])
        s_eps = singles.tile([P, 1], FP)
        nc.vector.memset(s_eps[:], EPS)

        # x state, split per t-half: [t_in, t_out(KH), d]
        s_x = [singles.tile([P, KH, D], FP, name=f"s_x{h}") for h in range(2)]
        s_xbf = [singles.tile([P, KH, D], BF, name=f"s_xbf{h}") for h in range(2)]
        s_xT = [singles.tile([P, DT, H], BF, name=f"s_xT{h}") for h in range(2)]

        for h in range(2):
            nc.sync.dma_start(
                s_x[h][:],
                d_x0[h * H:(h + 1) * H, :].rearrange("(o p) d -> p o d", p=P))
            nc.vector.tensor_copy(s_xbf[h][:], s_x[h][:])

        def transpose_x_half(h):
            for k in range(KH):
                for dc in range(DT):
                    tp = psT.tile([P, P], BF, tag="trbf")
                    nc.tensor.transpose(
                        tp[:], s_xbf[h][:, k, dc * P:(dc + 1) * P], ident_bf[:])
                    nc.scalar.copy(
                        s_xT[h][:, dc, k * P:(k + 1) * P], tp[:])

        transpose_x_half(0)
        transpose_x_half(1)

        def ln_batch(src_fn, nk):
            """Batched LN stats over nk row-tiles: returns (mean, rstd)."""
            s_mean = st_pool.tile([P, nk], FP, tag="mean")
            s_var = st_pool.tile([P, nk], FP, tag="var")
            s_rstd = st_pool.tile([P, nk], FP, tag="rstd")
            for kk in range(nk):
                stats = st_pool.tile([P, 6], FP, tag="bn")
                nc.vector.bn_stats(stats[:], src_fn(kk))
                mv = st_pool.tile([P, 2], FP, tag="mv")
                nc.vector.bn_aggr(mv[:], stats[:])
                nc.gpsimd.tensor_copy(s_mean[:, kk:kk + 1], mv[:, 0:1])
                nc.gpsimd.tensor_copy(s_var[:, kk:kk + 1], mv[:, 1:2])
            nc.scalar.activation(s_rstd[:], s_var[:], Sqrt, bias=s_eps[:])
            nc.vector.reciprocal(s_rstd[:], s_rstd[:])
            return s_mean, s_rstd

        # ================= layers =================
        def emit_layer(lid, prev_x1):
            if True:
                spill = dram.tile([NSH, T], BF, tag="spill", name=f"sp_{lid}")
                qr = [[None, None] for _ in range(NT)]
                xy = [[None, None] for _ in range(NT)]
                s_scT = [None, None]
                s_ykvT = [None, None]
                ykv_out_h = [None, None]
                ymlp_out_h = [None, None]

                def fetch_w(src, q, cache, nm):
                    if q not in cache:
                        wq = w_pool.tile([P, DT, 8 * P], BF, tag="w",
                                         name=f"{nm}_{q}")
                        nc.sync.dma_start(
                            wq[:], src[:, :, q * 8 * P:(q + 1) * 8 * P])
                        cache[q] = wq
                    return cache[q]

                def phase_A(th):
                    t0 = th * H
                    encb, trigb = {}, {}
                    for pi in range(FT):
                        stg = stage.tile([P, 2, H], BF, tag="stage",
                                         name=f"stg_{lid}_{th}_{pi}")
                        for eo, idx in enumerate((pi, FT + pi)):
                            wq = fetch_w(d_enc, idx // 8, encb,
                                         f"enc_{lid}_{th}")
                            col = (idx % 8) * P
                            ps = psA.tile([P, H], FP, tag="mm")
                            for dc in range(DT):
                                nc.tensor.matmul(
                                    ps[:], wq[:, dc, col:col + P],
                                    s_xT[th][:, dc, :],
                                    start=(dc == 0), stop=(dc == DT - 1))
                            nc.scalar.activation(stg[:, eo, :], ps[:], Relu)
                        nc.sync.dma_start(
                            spill[pi * 2 * P:(pi + 1) * 2 * P, t0:t0 + H]
                            .rearrange("(o p) t -> p o t", p=P), stg[:])
                        q = pi // 2
                        if q not in trigb:
                            ct = trig_pool.tile([P, 2, H], BF, tag="trig",
                                                name=f"cs_{lid}_{th}_{q}")
                            nc.sync.dma_start(
                                ct[:],
                                d_cos[q * 2 * P:(q + 1) * 2 * P, t0:t0 + H]
                                .rearrange("(o p) t -> p o t", p=P))
                            sn = trig_pool.tile([P, 2, H], BF, tag="trig",
                                                name=f"sn_{lid}_{th}_{q}")
                            nc.sync.dma_start(
                                sn[:],
                                d_sin[q * 2 * P:(q + 1) * 2 * P, t0:t0 + H]
                                .rearrange("(o p) t -> p o t", p=P))
                            trigb[q] = (ct, sn)
                        ce = trigb[q][0][:, pi % 2, :]
                        se = trigb[q][1][:, pi % 2, :]
                        E, O = stg[:, 0, :], stg[:, 1, :]
                        qe = big.tile([P, H], BF, tag="big",
                                      name=f"qe_{lid}_{th}_{pi}")
                        qo = big.tile([P, H], BF, tag="big",
                                      name=f"qo_{lid}_{th}_{pi}")
                        t2 = tmp_pool.tile([P, H], BF, tag="t2")
                        t4 = tmp_pool.tile([P, H], BF, tag="t4")
                        nc.vector.tensor_tensor(qe[:], E, ce, MUL)
                        nc.vector.tensor_tensor(t2[:], O, se, MUL)
                        nc.vector.tensor_tensor(qe[:], qe[:], t2[:], SUB)
                        nc.vector.tensor_tensor(qo[:], O, ce, MUL)
                        nc.gpsimd.tensor_tensor(t4[:], E, se, MUL)
                        nc.gpsimd.tensor_tensor(qo[:], qo[:], t4[:], ADD)
                        qr[pi][th], qr[FT + pi][th] = qe, qo

                def phase_B(th):
                    t0 = th * H
                    sct = big2.tile([P, KT, H], BF, tag="scT",
                                    name=f"scT_{lid}_{th}")
                    s_scT[th] = sct
                    for i_s in range(4 * th + 4):
                        hs, cs = i_s // 4, (i_s % 4) * P
                        ps = psA.tile([P, H], FP, tag="mm")
                        for n in range(NT):
                            nc.tensor.matmul(
                                ps[:], qr[n][hs][:, cs:cs + P], qr[n][th][:],
                                start=(n == 0), stop=(n == NT - 1))
                        dst = sct[:, i_s, :]
                        c0 = i_s * P - t0
                        if 0 <= c0 < H:
                            nc.vector.tensor_tensor(
                                dst[:, c0:c0 + P], ps[:, c0:c0 + P],
                                s_mask[:], MUL)
                            if c0 + P < H:
                                nc.vector.tensor_copy(
                                    dst[:, c0 + P:], ps[:, c0 + P:])
                        else:
                            nc.vector.tensor_copy(dst[:], ps[:])

                def phase_C(th):
                    ykv_in = dram.tile([H, D], BF, tag="ykv_in",
                                       name=f"yki_{lid}_{th}")
                    ykv_out = dram.tile([H, D], BF, tag="ykv_out",
                                        name=f"yko_{lid}_{th}")
                    kvst = ar_pool.tile([P, KH, D], BF, tag="arst",
                                        name=f"kvst_{lid}_{th}")
                    for kk in range(KH):
                        k = KH * th + kk
                        ps = psB.tile([P, D], FP, tag="pb")
                        for i in range(k + 1):
                            nc.tensor.matmul(
                                ps[:], s_scT[th][:, i, kk * P:(kk + 1) * P],
                                s_xbf[i // KH][:, i % KH, :],
                                start=(i == 0), stop=(i == k))
                        nc.vector.tensor_copy(kvst[:, kk, :], ps[:])
                    nc.sync.dma_start(
                        ykv_in.rearrange("(o p) d -> p o d", p=P), kvst[:])
                    nc.gpsimd.collective_compute(
                        "AllReduce", ADD,
                        replica_groups=[[0, 1], [2, 3], [4, 5], [6, 7]],
                        ins=[ykv_in.opt()], outs=[ykv_out.opt()])
                    ykv_out_h[th] = ykv_out

                def phase_D(th):
                    s_ykv = kv_pool.tile([P, KH, D], BF, tag="ykv",
                                         name=f"ykv_{lid}_{th}")
                    nc.sync.dma_start(
                        s_ykv[:],
                        ykv_out_h[th].rearrange("(o p) d -> p o d", p=P))
                    s_ykvn = kv_pool.tile([P, KH, D], BF, tag="ykvn",
                                          name=f"ykvn_{lid}_{th}")
                    ykvT = kv_pool.tile([P, DT, H], BF, tag="ykvT",
                                        name=f"ykvT_{lid}_{th}")
                    s_ykvT[th] = ykvT
                    mean, rstd = ln_batch(lambda kk: s_ykv[:, kk, :], KH)
                    for kk in range(KH):
                        nc.vector.tensor_scalar(
                            s_ykvn[:, kk, :], s_ykv[:, kk, :],
                            mean[:, kk:kk + 1], rstd[:, kk:kk + 1], SUB, MUL)
                        for dc in range(DT):
                            tp = psT.tile([P, P], BF, tag="trbf")
                            nc.tensor.transpose(
                                tp[:], s_ykvn[:, kk, dc * P:(dc + 1) * P],
                                ident_bf[:])
                            nc.scalar.copy(
                                ykvT[:, dc, kk * P:(kk + 1) * P], tp[:])

                def phase_Y(th):
                    t0 = th * H
                    encvb = {}
                    for pi in range(FT):
                        xr = stage.tile([P, 2, H], BF, tag="stage",
                                        name=f"xr_{lid}_{th}_{pi}")
                        nc.sync.dma_start(
                            xr[:],
                            spill[pi * 2 * P:(pi + 1) * 2 * P, t0:t0 + H]
                            .rearrange("(o p) t -> p o t", p=P))
                        for eo, idx in enumerate((pi, FT + pi)):
                            wq = fetch_w(d_encv, idx // 8, encvb,
                                         f"envv_{lid}_{th}")
                            col = (idx % 8) * P
                            xy_i = big.tile([P, H], BF, tag="big",
                                            name=f"xy_{lid}_{th}_{idx}")
                            ps = psA.tile([P, H], FP, tag="mm")
                            for dc in range(DT):
                                nc.tensor.matmul(
                                    ps[:], wq[:, dc, col:col + P],
                                    s_ykvT[th][:, dc, :],
                                    start=(dc == 0), stop=(dc == DT - 1))
                            yt = y_pool.tile([P, H], BF, tag="yt")
                            nc.scalar.activation(yt[:], ps[:], Relu)
                            nc.vector.tensor_tensor(
                                xy_i[:], xr[:, eo, :], yt[:], MUL)
                            xy[idx][th] = xy_i

                def phase_M(th):
                    ymlp_in = dram.tile([D, H], BF, tag="ymlp_in",
                                        name=f"ymi_{lid}_{th}")
                    ymlp_out = drsh.tile([D, H], BF, tag="ymlp_out",
                                         name=f"ymo_{lid}_{th}",
                                         addr_space="Shared")
                    decb = {}

                    def fetch_dec(q):
                        if q not in decb:
                            dq = w_pool.tile([P, 8, D], BF, tag="w",
                                             name=f"dec_{lid}_{th}_{q}")
                            nc.sync.dma_start(
                                dq[:], d_dec[:, q * 8:(q + 1) * 8, :])
                            decb[q] = dq
                        return decb[q]

                    pss = [psA.tile([P, H], FP, tag="mm",
                                    name=f"pss_{lid}_{th}_{dc}")
                           for dc in range(DT)]
                    for i in range(NT):
                        dq = fetch_dec(i // 8)
                        for dc in range(DT):
                            nc.tensor.matmul(
                                pss[dc][:], dq[:, i % 8, dc * P:(dc + 1) * P],
                                xy[i][th][:],
                                start=(i == 0), stop=(i == NT - 1))
                    ymst = ar_pool.tile([P, DT, H], BF, tag="arst",
                                        name=f"ymst_{lid}_{th}")
                    for dc in range(DT):
                        nc.vector.tensor_copy(ymst[:, dc, :], pss[dc][:])
                    nc.sync.dma_start(
                        ymlp_in.rearrange("(o p) t -> p o t", p=P), ymst[:])
                    nc.gpsimd.collective_compute(
                        "AllReduce", ADD,
                        replica_groups=[[0, 1, 2, 3, 4, 5, 6, 7]],
                        ins=[ymlp_in.opt()], outs=[ymlp_out.opt()])
                    ymlp_out_h[th] = ymlp_out

                def phase_X(th):
                    s_ymT = kv_pool.tile([P, DT, H], BF, tag="ymT",
                                         name=f"ymT_{lid}_{th}")
                    nc.sync.dma_start(
                        s_ymT[:],
                        ymlp_out_h[th].rearrange("(o p) t -> p o t", p=P))
                    s_ym = kv_pool.tile([P, KH, D], FP, tag="ym",
                                        name=f"ym_{lid}_{th}")
                    for kk in range(KH):
                        for dc in range(DT):
                            tp = psT.tile([P, P], BF, tag="trbf")
                            nc.tensor.transpose(
                                tp[:], s_ymT[:, dc, kk * P:(kk + 1) * P],
                                ident_bf[:])
                            nc.vector.tensor_copy(
                                s_ym[:, kk, dc * P:(dc + 1) * P], tp[:])
                    mean1, rstd1 = ln_batch(lambda kk: s_ym[:, kk, :], KH)
                    us = []
                    for kk in range(KH):
                        u = u_pool.tile([P, D], FP, tag="u",
                                        name=f"u_{lid}_{th}_{kk}")
                        nc.vector.tensor_scalar(
                            u[:], s_ym[:, kk, :], mean1[:, kk:kk + 1],
                            rstd1[:, kk:kk + 1], SUB, MUL)
                        nc.vector.tensor_tensor(u[:], u[:], s_x[th][:, kk, :],
                                                ADD)
                        us.append(u)
                    mean2, rstd2 = ln_batch(lambda kk: us[kk][:], KH)
                    for kk in range(KH):
                        nc.vector.tensor_scalar(
                            s_x[th][:, kk, :], us[kk][:], mean2[:, kk:kk + 1],
                            rstd2[:, kk:kk + 1], SUB, MUL)
                        nc.vector.tensor_copy(s_xbf[th][:, kk, :],
                                              s_x[th][:, kk, :])
                    transpose_x_half(th)

                phase_A(0)
                phase_B(0)
                phase_A(1)
                phase_C(0)
                phase_B(1)
                phase_C(1)
                phase_D(0)
                phase_Y(0)
                phase_M(0)
                phase_D(1)
                phase_Y(1)
                phase_M(1)
                phase_X(0)
                phase_X(1)

        for rep in range(reps):
            for layer in range(NL):
                emit_layer(rep * NL + layer, None)

        # ================= final head (fp32) =================
        s_xTf = big2.tile([P, DT, T], FP, tag="scT", name="xTf")
        for h in range(2):
            for kk in range(KH):
                for dc in range(DT):
                    tp = psB.tile([P, D], FP, tag="pb")
                    nc.tensor.transpose(
                        tp[:, :P], s_x[h][:, kk, dc * P:(dc + 1) * P],
                        ident_f[:])
                    nc.vector.tensor_copy(
                        s_xTf[:, dc, (h * KH + kk) * P:(h * KH + kk + 1) * P],
                        tp[:, :P])
        for k in range(KT):
            ps = psB.tile([P, VOCAB], FP, tag="pb")
            for dc in range(DT):
                nc.tensor.matmul(
                    ps[:], s_xTf[:, dc, k * P:(k + 1) * P], s_lmh[:, dc, :],
                    start=(dc == 0), stop=(dc == DT - 1))
            cp = cp_pool.tile([P, VOCAB], FP, tag="cpo", name=f"cpo_{k}")
            nc.vector.tensor_copy(cp[:], ps[:])
            nc.sync.dma_start(d_out[k * P:(k + 1) * P, :], cp[:])

    nc.compile()
    return nc


# --------------------------------------------------------------------------
# host side
# --------------------------------------------------------------------------

def _ln_np(x, eps=EPS):
    m = x.mean(-1, keepdims=True)
    v = ((x - m) ** 2).mean(-1, keepdims=True)
    return (x - m) / np.sqrt(v + eps)


def _prep_in_maps(inputs):
    idx = np.asarray(inputs["idx"]).reshape(-1).astype(np.int64)
    embed_w = np.asarray(inputs["embed_w"], dtype=np.float32)
    encoder = np.asarray(inputs["encoder"], dtype=np.float32)
    encoder_v = np.asarray(inputs["encoder_v"], dtype=np.float32)
    decoder = np.asarray(inputs["decoder"], dtype=np.float32).reshape(NH, N, D)
    lm_head = np.asarray(inputs["lm_head"], dtype=np.float32)

    x0 = _ln_np(embed_w[idx]).astype(np.float32)

    t_ = np.arange(N, dtype=np.float32)
    q = np.floor(t_ / np.float32(2.0)) * np.float32(2.0)
    freqs = (np.float32(1.0) / (np.float32(THETA) ** (q / np.float32(N)))
             / np.float32(TWO_PI))
    tt = np.arange(T, dtype=np.float32)
    phases = tt[:, None] * freqs[None, :]
    ph = np.mod(phases, np.float32(1.0)).astype(np.float32) * np.float32(TWO_PI)
    cos_full = np.cos(ph).astype(np.float32)
    sin_full = np.sin(ph).astype(np.float32)

    maskT = (np.arange(P)[:, None] < np.arange(P)[None, :]).astype(bf16)
    lmh = np.ascontiguousarray(
        lm_head.reshape(DT, P, VOCAB).transpose(1, 0, 2)).astype(np.float32)

    in_maps = []
    for c in range(8):
        h, half = c // 2, c % 2
        base = half * NSH
        perm = np.concatenate(
            [base + 2 * np.arange(F), base + 2 * np.arange(F) + 1])
        enc_c = encoder[h][:, perm]
        encv_c = encoder_v[h][:, perm]
        dec_c = decoder[h][perm, :]
        pcols = 2 * (half * F + np.arange(F))
        in_maps.append({
            "x0": x0,
            "cosT": np.ascontiguousarray(cos_full[:, pcols].T).astype(bf16),
            "sinT": np.ascontiguousarray(sin_full[:, pcols].T).astype(bf16),
            "enc": np.ascontiguousarray(
                enc_c.reshape(DT, P, NSH).transpose(1, 0, 2)).astype(bf16),
            "encv": np.ascontiguousarray(
                encv_c.reshape(DT, P, NSH).transpose(1, 0, 2)).astype(bf16),
            "dec": np.ascontiguousarray(
                dec_c.reshape(NT, P, D).transpose(1, 0, 2)).astype(bf16),
            "maskT": maskT,
            "lmh": lmh,
        })
    return in_maps


def kernel(**inputs) -> np.ndarray:
    in_maps = _prep_in_maps(inputs)
    if "nc" not in _CACHE:
        _CACHE["nc"] = _build_nc()
    res = run_bass_kernel_spmd(_CACHE["nc"], in_maps, core_ids=list(range(8)))
    logits = np.asarray(res.results[0]["logits"], dtype=np.float32)
    return logits.reshape(1, T, VOCAB)


# revision 15
# speedup vs baseline: 44.6952x; 1.0717x over previous
"""BDH forward (nn_BDH_4406636445721) on 8 TRN2 NeuronCores via Bass/Tile.

Sharding: core c -> head h=c//2, n-half=c%2 (4096 of 8192 latent rows).
Within a core's n-slice, n is permuted to [evens; odds] so RoPE pairs
(2i, 2i+1) become rows i (E block) and 2048+i (O block); encoder /
encoder_v columns and decoder rows get the same permutation, which is
transparent to every contraction over n.

The layer is software-pipelined over t-HALVES (t is never a contraction
dim, and causality means t-half0 only ever needs s<512): every AllReduce
(pair-AR of ykv partials, 8-core AR of ymlpT partials, both bf16)
overlaps compute of the other half / next layer.

Per layer and t-half (big tensors stored transposed [n, t]):
  x_spT = relu(enc^T @ xT)                  PE, spilled to HBM
  qrT   = rope(x_spT)                       DVE+GPSIMD
  scoresT[s,t] = sum_n qrT qrT              PE, upper-tri blocks only
  ykv_partial = masked scoresT contraction  PE -> pair AllReduce (ykv is
                                            linear in scores)
  ykvn = LN(ykv) -> ykvT                    DVE + PE transpose
  y_spT = relu(encv^T @ ykvT)               PE
  xyT   = x_spT * y_spT                     DVE
  ymlpT_partial = dec-stationary contract   PE -> 8-core AllReduce
  x = LN(x + LN(ymlp))                      DVE/ACT, replicated
Final: logits = x @ lm_head in fp32.
"""
import math
from contextlib import ExitStack

import numpy as np
import ml_dtypes

import concourse.bass as bass
import concourse.tile as tile
from concourse import bacc, mybir
from concourse.bass_utils import run_bass_kernel_spmd
from concourse.masks import make_identity

P = 128
T, D, NH, N, NL, VOCAB = 1024, 256, 4, 8192, 6, 256
NSH = N // 2            # n rows per core
F = NSH // 2            # rope pairs per core
NT = NSH // P           # 32 n-tiles
FT = F // P             # 16 pair-tiles (E tiles)
KT = T // P             # 8 t-tiles
DT = D // P             # 2 d-subtiles
H = 512                 # t-half width
KH = KT // 2            # t-tiles per half
THETA = 2.0 ** 16
TWO_PI = 2.0 * math.pi
EPS = 1e-5

BF = mybir.dt.bfloat16
FP = mybir.dt.float32
bf16 = ml_dtypes.bfloat16

_CACHE = {}


def _build_nc(reps=1, comm=True):
    nc = bacc.Bacc("TRN2", target_bir_lowering=False, debug=False, num_devices=8)

    d_x0 = nc.dram_tensor("x0", [T, D], FP, kind="ExternalInput")
    d_cos = nc.dram_tensor("cosT", [F, T], BF, kind="ExternalInput")
    d_sin = nc.dram_tensor("sinT", [F, T], BF, kind="ExternalInput")
    d_enc = nc.dram_tensor("enc", [P, DT, NSH], BF, kind="ExternalInput")
    d_encv = nc.dram_tensor("encv", [P, DT, NSH], BF, kind="ExternalInput")
    d_dec = nc.dram_tensor("dec", [P, NT, D], BF, kind="ExternalInput")
    d_mask = nc.dram_tensor("maskT", [P, P], BF, kind="ExternalInput")
    d_lmh = nc.dram_tensor("lmh", [P, DT, VOCAB], FP, kind="ExternalInput")
    d_out = nc.dram_tensor("logits", [T, VOCAB], FP, kind="ExternalOutput")

    Relu = mybir.ActivationFunctionType.Relu
    Sqrt = mybir.ActivationFunctionType.Sqrt
    MUL = mybir.AluOpType.mult
    SUB = mybir.AluOpType.subtract
    ADD = mybir.AluOpType.add

    with ExitStack() as ctx:
        tc = ctx.enter_context(tile.TileContext(nc))
        singles = ctx.enter_context(tc.tile_pool(name="singles", bufs=1))
        big = ctx.enter_context(tc.tile_pool(name="big", bufs=2 * NT))
        big2 = ctx.enter_context(tc.tile_pool(name="big2", bufs=2))
        stage = ctx.enter_context(tc.tile_pool(name="stage", bufs=4))
        tmp_pool = ctx.enter_context(tc.tile_pool(name="tmp", bufs=2))
        trig_pool = ctx.enter_context(tc.tile_pool(name="trig", bufs=4))
        w_pool = ctx.enter_context(tc.tile_pool(name="w", bufs=4))
        y_pool = ctx.enter_context(tc.tile_pool(name="y", bufs=3))
        cp_pool = ctx.enter_context(tc.tile_pool(name="cp", bufs=3))
        ar_pool = ctx.enter_context(tc.tile_pool(name="arst", bufs=3))
        st_pool = ctx.enter_context(tc.tile_pool(name="st", bufs=3))
        u_pool = ctx.enter_context(tc.tile_pool(name="u", bufs=KH))
        kv_pool = ctx.enter_context(tc.tile_pool(name="kv", bufs=2))
        psA = ctx.enter_context(tc.tile_pool(name="psA", bufs=4, space="PSUM"))
        psB = ctx.enter_context(tc.tile_pool(name="psB", bufs=2, space="PSUM"))
        psT = ctx.enter_context(tc.tile_pool(name="psT", bufs=2, space="PSUM"))
        dram = ctx.enter_context(tc.tile_pool(name="dram", bufs=2, space="DRAM"))
        drsh = ctx.enter_context(tc.tile_pool(name="drsh", bufs=4, space="DRAM"))

        # ---- persistent state ----
        s_mask = singles.tile([P, P], BF)
        nc.sync.dma_start(s_mask[:], d_mask[:])
        s_lmh = singles.tile([P, DT, VOCAB], FP)
        nc.sync.dma_start(s_lmh[:], d_lmh[:])
        ident_bf = singles.tile([P, P], BF)
        make_identity(nc, ident_bf[:])
        ident_f = singles.tile([P, P], FP)
        make_identity(nc, ident_f[:])
        s_eps = singles.tile([P, 1], FP)
        nc.vector.memset(s_eps[:], EPS)

        # x state, split per t-half: [t_in, t_out(KH), d]
        s_x = [singles.tile([P, KH, D], FP, name=f"s_x{h}") for h in range(2)]
        s_xbf = [singles.tile([P, KH, D], BF, name=f"s_xbf{h}") for h in range(2)]
        s_xT = [singles.tile([P, DT, H], BF, name=f"s_xT{h}") for h in range(2)]

        for h in range(2):
            nc.sync.dma_start(
                s_x[h][:],
                d_x0[h * H:(h + 1) * H, :].rearrange("(o p) d -> p o d", p=P))
            nc.vector.tensor_copy(s_xbf[h][:], s_x[h][:])

        def transpose_x_half(h):
            for k in range(KH):
                for dc in range(DT):
                    tp = psT.tile([P, P], BF, tag="trbf")
                    nc.tensor.transpose(
                        tp[:], s_xbf[h][:, k, dc * P:(dc + 1) * P], ident_bf[:])
                    nc.scalar.copy(
                        s_xT[h][:, dc, k * P:(k + 1) * P], tp[:])

        transpose_x_half(0)
        transpose_x_half(1)

        def ln_batch(src_fn, nk):
            """Batched LN stats over nk row-tiles: returns (mean, rstd)."""
            s_mean = st_pool.tile([P, nk], FP, tag="mean")
            s_var = st_pool.tile([P, nk], FP, tag="var")
            s_rstd = st_pool.tile([P, nk], FP, tag="rstd")
            for kk in range(nk):
                stats = st_pool.tile([P, 6], FP, tag="bn")
                nc.vector.bn_stats(stats[:], src_fn(kk))
                mv = st_pool.tile([P, 2], FP, tag="mv")
                nc.vector.bn_aggr(mv[:], stats[:])
                nc.gpsimd.tensor_copy(s_mean[:, kk:kk + 1], mv[:, 0:1])
                nc.gpsimd.tensor_copy(s_var[:, kk:kk + 1], mv[:, 1:2])
            nc.scalar.activation(s_rstd[:], s_var[:], Sqrt, bias=s_eps[:])
            nc.vector.reciprocal(s_rstd[:], s_rstd[:])
            return s_mean, s_rstd

        # ================= layers =================
        def emit_layer(lid, prev_x1):
            if True:
                spill = dram.tile([NSH, T], BF, tag="spill", name=f"sp_{lid}")
                qr = [[None, None] for _ in range(NT)]
                xy = [[None, None] for _ in range(NT)]
                s_scT = [None, None]
                s_ykvT = [None, None]
                ykv_out_h = [None, None]
                ymlp_out_h = [None, None]

                def fetch_w(src, q, cache, nm):
                    if q not in cache:
                        wq = w_pool.tile([P, DT, 8 * P], BF, tag="w",
                                         name=f"{nm}_{q}")
                        nc.sync.dma_start(
                            wq[:], src[:, :, q * 8 * P:(q + 1) * 8 * P])
                        cache[q] = wq
                    return cache[q]

                def phase_A(th):
                    t0 = th * H
                    encb, trigb = {}, {}
                    for pi in range(FT):
                        stg = stage.tile([P, 2, H], BF, tag="stage",
                                         name=f"stg_{lid}_{th}_{pi}")
                        for eo, idx in enumerate((pi, FT + pi)):
                            wq = fetch_w(d_enc, idx // 8, encb,
                                         f"enc_{lid}_{th}")
                            col = (idx % 8) * P
                            ps = psA.tile([P, H], FP, tag="mm")
                            for dc in range(DT):
                                nc.tensor.matmul(
                                    ps[:], wq[:, dc, col:col + P],
                                    s_xT[th][:, dc, :],
                                    start=(dc == 0), stop=(dc == DT - 1))
                            nc.scalar.activation(stg[:, eo, :], ps[:], Relu)
                        nc.sync.dma_start(
                            spill[pi * 2 * P:(pi + 1) * 2 * P, t0:t0 + H]
                            .rearrange("(o p) t -> p o t", p=P), stg[:])
                        q = pi // 2
                        if q not in trigb:
                            ct = trig_pool.tile([P, 2, H], BF, tag="trig",
                                                name=f"cs_{lid}_{th}_{q}")
                            nc.sync.dma_start(
                                ct[:],
                                d_cos[q * 2 * P:(q + 1) * 2 * P, t0:t0 + H]
                                .rearrange("(o p) t -> p o t", p=P))
                            sn = trig_pool.tile([P, 2, H], BF, tag="trig",
                                                name=f"sn_{lid}_{th}_{q}")
                            nc.sync.dma_start(
                                sn[:],
                                d_sin[q * 2 * P:(q + 1) * 2 * P, t0:t0 + H]
                                .rearrange("(o p) t -> p o t", p=P))
                            trigb[q] = (ct, sn)
                        ce = trigb[q][0][:, pi % 2, :]
                        se = trigb[q][1][:, pi % 2, :]
                        E, O = stg[:, 0, :], stg[:, 1, :]
                        qe = big.tile([P, H], BF, tag="big",
                                      name=f"qe_{lid}_{th}_{pi}")
                        qo = big.tile([P, H], BF, tag="big",
                                      name=f"qo_{lid}_{th}_{pi}")
                        t2 = tmp_pool.tile([P, H], BF, tag="t2")
                        t4 = tmp_pool.tile([P, H], BF, tag="t4")
                        nc.vector.tensor_tensor(qe[:], E, ce, MUL)
                        nc.vector.tensor_tensor(t2[:], O, se, MUL)
                        nc.vector.tensor_tensor(qe[:], qe[:], t2[:], SUB)
                        nc.vector.tensor_tensor(qo[:], O, ce, MUL)
                        nc.gpsimd.tensor_tensor(t4[:], E, se, MUL)
                        nc.gpsimd.tensor_tensor(qo[:], qo[:], t4[:], ADD)
                        qr[pi][th], qr[FT + pi][th] = qe, qo

                def phase_B(th):
                    t0 = th * H
                    sct = big2.tile([P, KT, H], BF, tag="scT",
                                    name=f"scT_{lid}_{th}")
                    s_scT[th] = sct
                    for i_s in range(4 * th + 4):
                        hs, cs = i_s // 4, (i_s % 4) * P
                        ps = psA.tile([P, H], FP, tag="mm")
                        for n in range(NT):
                            nc.tensor.matmul(
                                ps[:], qr[n][hs][:, cs:cs + P], qr[n][th][:],
                                start=(n == 0), stop=(n == NT - 1))
                        dst = sct[:, i_s, :]
                        c0 = i_s * P - t0
                        if 0 <= c0 < H:
                            nc.vector.tensor_tensor(
                                dst[:, c0:c0 + P], ps[:, c0:c0 + P],
                                s_mask[:], MUL)
                            if c0 + P < H:
                                nc.vector.tensor_copy(
                                    dst[:, c0 + P:], ps[:, c0 + P:])
                        else:
                            nc.vector.tensor_copy(dst[:], ps[:])

                def phase_C(th):
                    ykv_in = dram.tile([H, D], BF, tag="ykv_in",
                                       name=f"yki_{lid}_{th}")
                    ykv_out = dram.tile([H, D], BF, tag="ykv_out",
                                        name=f"yko_{lid}_{th}")
                    kvst = ar_pool.tile([P, KH, D], BF, tag="arst",
                                        name=f"kvst_{lid}_{th}")
                    for kk in range(KH):
                        k = KH * th + kk
                        ps = psB.tile([P, D], FP, tag="pb")
                        for i in range(k + 1):
                            nc.tensor.matmul(
                                ps[:], s_scT[th][:, i, kk * P:(kk + 1) * P],
                                s_xbf[i // KH][:, i % KH, :],
                                start=(i == 0), stop=(i == k))
                        nc.vector.tensor_copy(kvst[:, kk, :], ps[:])
                    nc.sync.dma_start(
                        ykv_in.rearrange("(o p) d -> p o d", p=P), kvst[:])
                    if comm:
                        nc.gpsimd.collective_compute(
                            "AllReduce", ADD,
                            replica_groups=[[0, 1], [2, 3], [4, 5], [6, 7]],
                            ins=[ykv_in.opt()], outs=[ykv_out.opt()])
                    else:
                        nc.sync.dma_start(ykv_out.opt(), ykv_in.opt())
                    ykv_out_h[th] = ykv_out

                def phase_D(th):
                    s_ykv = kv_pool.tile([P, KH, D], BF, tag="ykv",
                                         name=f"ykv_{lid}_{th}")
                    nc.sync.dma_start(
                        s_ykv[:],
                        ykv_out_h[th].rearrange("(o p) d -> p o d", p=P))
                    s_ykvn = kv_pool.tile([P, KH, D], BF, tag="ykvn",
                                          name=f"ykvn_{lid}_{th}")
                    ykvT = kv_pool.tile([P, DT, H], BF, tag="ykvT",
                                        name=f"ykvT_{lid}_{th}")
                    s_ykvT[th] = ykvT
                    mean, rstd = ln_batch(lambda kk: s_ykv[:, kk, :], KH)
                    for kk in range(KH):
                        nc.vector.tensor_scalar(
                            s_ykvn[:, kk, :], s_ykv[:, kk, :],
                            mean[:, kk:kk + 1], rstd[:, kk:kk + 1], SUB, MUL)
                        for dc in range(DT):
                            tp = psT.tile([P, P], BF, tag="trbf")
                            nc.tensor.transpose(
                                tp[:], s_ykvn[:, kk, dc * P:(dc + 1) * P],
                                ident_bf[:])
                            nc.scalar.copy(
                                ykvT[:, dc, kk * P:(kk + 1) * P], tp[:])

                def phase_Y(th):
                    t0 = th * H
                    encvb = {}
                    for pi in range(FT):
                        xr = stage.tile([P, 2, H], BF, tag="stage",
                                        name=f"xr_{lid}_{th}_{pi}")
                        nc.sync.dma_start(
                            xr[:],
                            spill[pi * 2 * P:(pi + 1) * 2 * P, t0:t0 + H]
                            .rearrange("(o p) t -> p o t", p=P))
                        for eo, idx in enumerate((pi, FT + pi)):
                            wq = fetch_w(d_encv, idx // 8, encvb,
                                         f"envv_{lid}_{th}")
                            col = (idx % 8) * P
                            xy_i = big.tile([P, H], BF, tag="big",
                                            name=f"xy_{lid}_{th}_{idx}")
                            ps = psA.tile([P, H], FP, tag="mm")
                            for dc in range(DT):
                                nc.tensor.matmul(
                                    ps[:], wq[:, dc, col:col + P],
                                    s_ykvT[th][:, dc, :],
                                    start=(dc == 0), stop=(dc == DT - 1))
                            yt = y_pool.tile([P, H], BF, tag="yt")
                            nc.scalar.activation(yt[:], ps[:], Relu)
                            nc.vector.tensor_tensor(
                                xy_i[:], xr[:, eo, :], yt[:], MUL)
                            xy[idx][th] = xy_i

                def phase_M(th):
                    ymlp_in = dram.tile([D, H], BF, tag="ymlp_in",
                                        name=f"ymi_{lid}_{th}")
                    ymlp_out = drsh.tile([D, H], BF, tag="ymlp_out",
                                         name=f"ymo_{lid}_{th}",
                                         addr_space="Shared")
                    decb = {}

                    def fetch_dec(q):
                        if q not in decb:
                            dq = w_pool.tile([P, 8, D], BF, tag="w",
                                             name=f"dec_{lid}_{th}_{q}")
                            nc.sync.dma_start(
                                dq[:], d_dec[:, q * 8:(q + 1) * 8, :])
                            decb[q] = dq
                        return decb[q]

                    pss = [psA.tile([P, H], FP, tag="mm",
                                    name=f"pss_{lid}_{th}_{dc}")
                           for dc in range(DT)]
                    for i in range(NT):
                        dq = fetch_dec(i // 8)
                        for dc in range(DT):
                            nc.tensor.matmul(
                                pss[dc][:], dq[:, i % 8, dc * P:(dc + 1) * P],
                                xy[i][th][:],
                                start=(i == 0), stop=(i == NT - 1))
                    ymst = ar_pool.tile([P, DT, H], BF, tag="arst",
                                        name=f"ymst_{lid}_{th}")
                    for dc in range(DT):
                        nc.vector.tensor_copy(ymst[:, dc, :], pss[dc][:])
                    nc.sync.dma_start(
                        ymlp_in.rearrange("(o p) t -> p o t", p=P), ymst[:])
                    if comm:
                        nc.gpsimd.collective_compute(
                            "AllReduce", ADD,
                            replica_groups=[[0, 1, 2, 3, 4, 5, 6, 7]],
                            ins=[ymlp_in.opt()], outs=[ymlp_out.opt()])
                    else:
                        nc.sync.dma_start(ymlp_out.opt(), ymlp_in.opt())
                    ymlp_out_h[th] = ymlp_out

                def phase_X(th):
                    s_ymT = kv_pool.tile([P, DT, H], BF, tag="ymT",
                                         name=f"ymT_{lid}_{th}")
                    nc.sync.dma_start(
                        s_ymT[:],
                        ymlp_out_h[th].rearrange("(o p) t -> p o t", p=P))
                    s_ym = kv_pool.tile([P, KH, D], FP, tag="ym",
                                        name=f"ym_{lid}_{th}")
                    for kk in range(KH):
                        for dc in range(DT):
                            tp = psT.tile([P, P], BF, tag="trbf")
                            nc.tensor.transpose(
                                tp[:], s_ymT[:, dc, kk * P:(kk + 1) * P],
                                ident_bf[:])
                            nc.vector.tensor_copy(
                                s_ym[:, kk, dc * P:(dc + 1) * P], tp[:])
                    mean1, rstd1 = ln_batch(lambda kk: s_ym[:, kk, :], KH)
                    us = []
                    for kk in range(KH):
                        u = u_pool.tile([P, D], FP, tag="u",
                                        name=f"u_{lid}_{th}_{kk}")
                        nc.vector.tensor_scalar(
                            u[:], s_ym[:, kk, :], mean1[:, kk:kk + 1],
                            rstd1[:, kk:kk + 1], SUB, MUL)
                        nc.vector.tensor_tensor(u[:], u[:], s_x[th][:, kk, :],
                                                ADD)
                        us.append(u)
                    mean2, rstd2 = ln_batch(lambda kk: us[kk][:], KH)
                    for kk in range(KH):
                        nc.vector.tensor_scalar(
                            s_x[th][:, kk, :], us[kk][:], mean2[:, kk:kk + 1],
                            rstd2[:, kk:kk + 1], SUB, MUL)
                        nc.vector.tensor_copy(s_xbf[th][:, kk, :],
                                              s_x[th][:, kk, :])
                    transpose_x_half(th)

                phase_A(0)
                if prev_x1 is not None:
                    prev_x1()      # prev layer's X(1): overlaps its AR behind A0
                phase_B(0)
                phase_A(1)
                phase_C(0)
                phase_B(1)
                phase_C(1)
                phase_D(0)
                phase_Y(0)
                phase_M(0)
                phase_D(1)
                phase_Y(1)
                phase_M(1)
                phase_X(0)
                return lambda: phase_X(1)

        prev = None
        for rep in range(reps):
            for layer in range(NL):
                prev = emit_layer(rep * NL + layer, prev)
        if prev is not None:
            prev()

        # ================= final head (fp32) =================
        s_xTf = big2.tile([P, DT, T], FP, tag="scT", name="xTf")
        for h in range(2):
            for kk in range(KH):
                for dc in range(DT):
                    tp = psB.tile([P, D], FP, tag="pb")
                    nc.tensor.transpose(
                        tp[:, :P], s_x[h][:, kk, dc * P:(dc + 1) * P],
                        ident_f[:])
                    nc.vector.tensor_copy(
                        s_xTf[:, dc, (h * KH + kk) * P:(h * KH + kk + 1) * P],
                        tp[:, :P])
        for k in range(KT):
            ps = psB.tile([P, VOCAB], FP, tag="pb")
            for dc in range(DT):
                nc.tensor.matmul(
                    ps[:], s_xTf[:, dc, k * P:(k + 1) * P], s_lmh[:, dc, :],
                    start=(dc == 0), stop=(dc == DT - 1))
            cp = cp_pool.tile([P, VOCAB], FP, tag="cpo", name=f"cpo_{k}")
            nc.vector.tensor_copy(cp[:], ps[:])
            nc.sync.dma_start(d_out[k * P:(k + 1) * P, :], cp[:])

    nc.compile()
    return nc


# --------------------------------------------------------------------------
# host side
# --------------------------------------------------------------------------

def _ln_np(x, eps=EPS):
    m = x.mean(-1, keepdims=True)
    v = ((x - m) ** 2).mean(-1, keepdims=True)
    return (x - m) / np.sqrt(v + eps)


def _prep_in_maps(inputs):
    idx = np.asarray(inputs["idx"]).reshape(-1).astype(np.int64)
    embed_w = np.asarray(inputs["embed_w"], dtype=np.float32)
    encoder = np.asarray(inputs["encoder"], dtype=np.float32)
    encoder_v = np.asarray(inputs["encoder_v"], dtype=np.float32)
    decoder = np.asarray(inputs["decoder"], dtype=np.float32).reshape(NH, N, D)
    lm_head = np.asarray(inputs["lm_head"], dtype=np.float32)

    x0 = _ln_np(embed_w[idx]).astype(np.float32)

    t_ = np.arange(N, dtype=np.float32)
    q = np.floor(t_ / np.float32(2.0)) * np.float32(2.0)
    freqs = (np.float32(1.0) / (np.float32(THETA) ** (q / np.float32(N)))
             / np.float32(TWO_PI))
    tt = np.arange(T, dtype=np.float32)
    phases = tt[:, None] * freqs[None, :]
    ph = np.mod(phases, np.float32(1.0)).astype(np.float32) * np.float32(TWO_PI)
    cos_full = np.cos(ph).astype(np.float32)
    sin_full = np.sin(ph).astype(np.float32)

    maskT = (np.arange(P)[:, None] < np.arange(P)[None, :]).astype(bf16)
    lmh = np.ascontiguousarray(
        lm_head.reshape(DT, P, VOCAB).transpose(1, 0, 2)).astype(np.float32)

    in_maps = []
    for c in range(8):
        h, half = c // 2, c % 2
        base = half * NSH
        perm = np.concatenate(
            [base + 2 * np.arange(F), base + 2 * np.arange(F) + 1])
        enc_c = encoder[h][:, perm]
        encv_c = encoder_v[h][:, perm]
        dec_c = decoder[h][perm, :]
        pcols = 2 * (half * F + np.arange(F))
        in_maps.append({
            "x0": x0,
            "cosT": np.ascontiguousarray(cos_full[:, pcols].T).astype(bf16),
            "sinT": np.ascontiguousarray(sin_full[:, pcols].T).astype(bf16),
            "enc": np.ascontiguousarray(
                enc_c.reshape(DT, P, NSH).transpose(1, 0, 2)).astype(bf16),
            "encv": np.ascontiguousarray(
                encv_c.reshape(DT, P, NSH).transpose(1, 0, 2)).astype(bf16),
            "dec": np.ascontiguousarray(
                dec_c.reshape(NT, P, D).transpose(1, 0, 2)).astype(bf16),
            "maskT": maskT,
            "lmh": lmh,
        })
    return in_maps


def kernel(**inputs) -> np.ndarray:
    in_maps = _prep_in_maps(inputs)
    if "nc" not in _CACHE:
        _CACHE["nc"] = _build_nc()
    res = run_bass_kernel_spmd(_CACHE["nc"], in_maps, core_ids=list(range(8)))
    logits = np.asarray(res.results[0]["logits"], dtype=np.float32)
    return logits.reshape(1, T, VOCAB)


# revision 16
# speedup vs baseline: 48.8888x; 1.0938x over previous
"""BDH forward (nn_BDH_4406636445721) on 8 TRN2 NeuronCores via Bass/Tile.

Sharding: core c -> head h=c//2, n-half=c%2 (4096 of 8192 latent rows).
Within a core's n-slice, n is permuted to [evens; odds] so RoPE pairs
(2i, 2i+1) become rows i (E block) and 2048+i (O block); encoder /
encoder_v columns and decoder rows get the same permutation, which is
transparent to every contraction over n.

The layer is software-pipelined over t-HALVES (t is never a contraction
dim, and causality means t-half0 only ever needs s<512): every AllReduce
(pair-AR of ykv partials, 8-core AR of ymlpT partials, both bf16)
overlaps compute of the other half / next layer.

Per layer and t-half (big tensors stored transposed [n, t]):
  x_spT = relu(enc^T @ xT)                  PE, spilled to HBM
  qrT   = rope(x_spT)                       DVE+GPSIMD
  scoresT[s,t] = sum_n qrT qrT              PE, upper-tri blocks only
  ykv_partial = masked scoresT contraction  PE -> pair AllReduce (ykv is
                                            linear in scores)
  ykvn = LN(ykv) -> ykvT                    DVE + PE transpose
  y_spT = relu(encv^T @ ykvT)               PE
  xyT   = x_spT * y_spT                     DVE
  ymlpT_partial = dec-stationary contract   PE -> 8-core AllReduce
  x = LN(x + LN(ymlp))                      DVE/ACT, replicated
Final: logits = x @ lm_head in fp32.
"""
import math
from contextlib import ExitStack

import numpy as np
import ml_dtypes

import concourse.bass as bass
import concourse.tile as tile
from concourse import bacc, mybir
from concourse.bass_utils import run_bass_kernel_spmd
from concourse.masks import make_identity

P = 128
T, D, NH, N, NL, VOCAB = 1024, 256, 4, 8192, 6, 256
NSH = N // 2            # n rows per core
F = NSH // 2            # rope pairs per core
NT = NSH // P           # 32 n-tiles
FT = F // P             # 16 pair-tiles (E tiles)
KT = T // P             # 8 t-tiles
DT = D // P             # 2 d-subtiles
H = 512                 # t-half width
KH = KT // 2            # t-tiles per half
THETA = 2.0 ** 16
TWO_PI = 2.0 * math.pi
EPS = 1e-5

BF = mybir.dt.bfloat16
FP = mybir.dt.float32
bf16 = ml_dtypes.bfloat16

_CACHE = {}


def _build_nc(reps=1, comm=True):
    nc = bacc.Bacc("TRN2", target_bir_lowering=False, debug=False, num_devices=8)

    d_x0 = nc.dram_tensor("x0", [T, D], FP, kind="ExternalInput")
    d_cos = nc.dram_tensor("cosT", [F, T], BF, kind="ExternalInput")
    d_sin = nc.dram_tensor("sinT", [F, T], BF, kind="ExternalInput")
    d_enc = nc.dram_tensor("enc", [P, DT, NSH], BF, kind="ExternalInput")
    d_encv = nc.dram_tensor("encv", [P, DT, NSH], BF, kind="ExternalInput")
    d_dec = nc.dram_tensor("dec", [P, NT, D], BF, kind="ExternalInput")
    d_mask = nc.dram_tensor("maskT", [P, P], BF, kind="ExternalInput")
    d_lmh = nc.dram_tensor("lmh", [P, DT, VOCAB], FP, kind="ExternalInput")
    d_out = nc.dram_tensor("logits", [T, VOCAB], FP, kind="ExternalOutput")

    Relu = mybir.ActivationFunctionType.Relu
    Sqrt = mybir.ActivationFunctionType.Sqrt
    MUL = mybir.AluOpType.mult
    SUB = mybir.AluOpType.subtract
    ADD = mybir.AluOpType.add

    with ExitStack() as ctx:
        tc = ctx.enter_context(tile.TileContext(nc))
        singles = ctx.enter_context(tc.tile_pool(name="singles", bufs=1))
        big = ctx.enter_context(tc.tile_pool(name="big", bufs=2 * NT))
        big2 = ctx.enter_context(tc.tile_pool(name="big2", bufs=2))
        stage = ctx.enter_context(tc.tile_pool(name="stage", bufs=4))
        tmp_pool = ctx.enter_context(tc.tile_pool(name="tmp", bufs=2))
        trig_pool = ctx.enter_context(tc.tile_pool(name="trig", bufs=4))
        w_pool = ctx.enter_context(tc.tile_pool(name="w", bufs=4))
        y_pool = ctx.enter_context(tc.tile_pool(name="y", bufs=3))
        cp_pool = ctx.enter_context(tc.tile_pool(name="cp", bufs=3))
        ar_pool = ctx.enter_context(tc.tile_pool(name="arst", bufs=3))
        st_pool = ctx.enter_context(tc.tile_pool(name="st", bufs=3))
        u_pool = ctx.enter_context(tc.tile_pool(name="u", bufs=KH))
        kv_pool = ctx.enter_context(tc.tile_pool(name="kv", bufs=2))
        psA = ctx.enter_context(tc.tile_pool(name="psA", bufs=4, space="PSUM"))
        psB = ctx.enter_context(tc.tile_pool(name="psB", bufs=2, space="PSUM"))
        psT = ctx.enter_context(tc.tile_pool(name="psT", bufs=2, space="PSUM"))
        dram = ctx.enter_context(tc.tile_pool(name="dram", bufs=2, space="DRAM"))
        drsh = ctx.enter_context(tc.tile_pool(name="drsh", bufs=4, space="DRAM"))

        # ---- persistent state ----
        s_mask = singles.tile([P, P], BF)
        nc.sync.dma_start(s_mask[:], d_mask[:])
        s_lmh = singles.tile([P, DT, VOCAB], FP)
        nc.sync.dma_start(s_lmh[:], d_lmh[:])
        ident_bf = singles.tile([P, P], BF)
        make_identity(nc, ident_bf[:])
        ident_f = singles.tile([P, P], FP)
        make_identity(nc, ident_f[:])
        s_eps = singles.tile([P, 1], FP)
        nc.vector.memset(s_eps[:], EPS)

        # x state, split per t-half: [t_in, t_out(KH), d]
        s_x = [singles.tile([P, KH, D], FP, name=f"s_x{h}") for h in range(2)]
        s_xbf = [singles.tile([P, KH, D], BF, name=f"s_xbf{h}") for h in range(2)]
        s_xT = [singles.tile([P, DT, H], BF, name=f"s_xT{h}") for h in range(2)]

        for h in range(2):
            nc.sync.dma_start(
                s_x[h][:],
                d_x0[h * H:(h + 1) * H, :].rearrange("(o p) d -> p o d", p=P))
            nc.vector.tensor_copy(s_xbf[h][:], s_x[h][:])

        def transpose_x_half(h):
            for k in range(KH):
                for dc in range(DT):
                    tp = psT.tile([P, P], BF, tag="trbf")
                    nc.tensor.transpose(
                        tp[:], s_xbf[h][:, k, dc * P:(dc + 1) * P], ident_bf[:])
                    nc.scalar.copy(
                        s_xT[h][:, dc, k * P:(k + 1) * P], tp[:])

        transpose_x_half(0)
        transpose_x_half(1)

        def ln_batch(src_fn, nk):
            """Batched LN stats over nk row-tiles: returns (mean, rstd)."""
            s_mean = st_pool.tile([P, nk], FP, tag="mean")
            s_var = st_pool.tile([P, nk], FP, tag="var")
            s_rstd = st_pool.tile([P, nk], FP, tag="rstd")
            for kk in range(nk):
                stats = st_pool.tile([P, 6], FP, tag="bn")
                nc.vector.bn_stats(stats[:], src_fn(kk))
                mv = st_pool.tile([P, 2], FP, tag="mv")
                nc.vector.bn_aggr(mv[:], stats[:])
                nc.gpsimd.tensor_copy(s_mean[:, kk:kk + 1], mv[:, 0:1])
                nc.gpsimd.tensor_copy(s_var[:, kk:kk + 1], mv[:, 1:2])
            nc.scalar.activation(s_rstd[:], s_var[:], Sqrt, bias=s_eps[:])
            nc.vector.reciprocal(s_rstd[:], s_rstd[:])
            return s_mean, s_rstd

        # ================= layers =================
        def emit_layer(lid, prev_x1):
            if True:
                spill = dram.tile([NSH, T], BF, tag="spill", name=f"sp_{lid}")
                qr = [[None, None] for _ in range(NT)]
                xy = [[None, None] for _ in range(NT)]
                s_scT = [None, None]
                s_ykvT = [None, None]
                ykv_out_h = [None, None]
                ymlp_out_h = [None, None]

                def fetch_w(src, q, cache, nm):
                    if q not in cache:
                        wq = w_pool.tile([P, DT, 8 * P], BF, tag="w",
                                         name=f"{nm}_{q}")
                        nc.sync.dma_start(
                            wq[:], src[:, :, q * 8 * P:(q + 1) * 8 * P])
                        cache[q] = wq
                    return cache[q]

                def phase_A(th):
                    t0 = th * H
                    encb, trigb = {}, {}
                    stg = None
                    for pi in range(FT):
                        if pi % 2 == 0:
                            stg = stage.tile([P, 4, H], BF, tag="stage",
                                             name=f"stg_{lid}_{th}_{pi}")
                        so = (pi % 2) * 2
                        for eo, idx in enumerate((pi, FT + pi)):
                            wq = fetch_w(d_enc, idx // 8, encb,
                                         f"enc_{lid}_{th}")
                            col = (idx % 8) * P
                            ps = psA.tile([P, H], FP, tag="mm")
                            for dc in range(DT):
                                nc.tensor.matmul(
                                    ps[:], wq[:, dc, col:col + P],
                                    s_xT[th][:, dc, :],
                                    start=(dc == 0), stop=(dc == DT - 1))
                            nc.scalar.activation(stg[:, so + eo, :], ps[:], Relu)
                        if pi % 2 == 1:
                            nc.sync.dma_start(
                                spill[(pi - 1) * 2 * P:(pi + 1) * 2 * P,
                                      t0:t0 + H]
                                .rearrange("(o p) t -> p o t", p=P), stg[:])
                        q = pi // 2
                        if q not in trigb:
                            ct = trig_pool.tile([P, 2, H], BF, tag="trig",
                                                name=f"cs_{lid}_{th}_{q}")
                            nc.sync.dma_start(
                                ct[:],
                                d_cos[q * 2 * P:(q + 1) * 2 * P, t0:t0 + H]
                                .rearrange("(o p) t -> p o t", p=P))
                            sn = trig_pool.tile([P, 2, H], BF, tag="trig",
                                                name=f"sn_{lid}_{th}_{q}")
                            nc.sync.dma_start(
                                sn[:],
                                d_sin[q * 2 * P:(q + 1) * 2 * P, t0:t0 + H]
                                .rearrange("(o p) t -> p o t", p=P))
                            trigb[q] = (ct, sn)
                        ce = trigb[q][0][:, pi % 2, :]
                        se = trigb[q][1][:, pi % 2, :]
                        E, O = stg[:, so, :], stg[:, so + 1, :]
                        qe = big.tile([P, H], BF, tag="big",
                                      name=f"qe_{lid}_{th}_{pi}")
                        qo = big.tile([P, H], BF, tag="big",
                                      name=f"qo_{lid}_{th}_{pi}")
                        t2 = tmp_pool.tile([P, H], BF, tag="t2")
                        t4 = tmp_pool.tile([P, H], BF, tag="t4")
                        nc.vector.tensor_tensor(qe[:], E, ce, MUL)
                        nc.vector.tensor_tensor(t2[:], O, se, MUL)
                        nc.vector.tensor_tensor(qe[:], qe[:], t2[:], SUB)
                        nc.vector.tensor_tensor(qo[:], O, ce, MUL)
                        nc.gpsimd.tensor_tensor(t4[:], E, se, MUL)
                        nc.gpsimd.tensor_tensor(qo[:], qo[:], t4[:], ADD)
                        qr[pi][th], qr[FT + pi][th] = qe, qo

                def phase_B(th):
                    t0 = th * H
                    sct = big2.tile([P, KT, H], BF, tag="scT",
                                    name=f"scT_{lid}_{th}")
                    s_scT[th] = sct
                    for i_s in range(4 * th + 4):
                        hs, cs = i_s // 4, (i_s % 4) * P
                        ps = psA.tile([P, H], FP, tag="mm")
                        for n in range(NT):
                            nc.tensor.matmul(
                                ps[:], qr[n][hs][:, cs:cs + P], qr[n][th][:],
                                start=(n == 0), stop=(n == NT - 1))
                        dst = sct[:, i_s, :]
                        c0 = i_s * P - t0
                        if 0 <= c0 < H:
                            nc.vector.tensor_tensor(
                                dst[:, c0:c0 + P], ps[:, c0:c0 + P],
                                s_mask[:], MUL)
                            if c0 + P < H:
                                nc.vector.tensor_copy(
                                    dst[:, c0 + P:], ps[:, c0 + P:])
                        else:
                            nc.vector.tensor_copy(dst[:], ps[:])

                def phase_C(th):
                    ykv_in = dram.tile([H, D], BF, tag="ykv_in",
                                       name=f"yki_{lid}_{th}")
                    ykv_out = dram.tile([H, D], BF, tag="ykv_out",
                                        name=f"yko_{lid}_{th}")
                    kvst = ar_pool.tile([P, KH, D], BF, tag="arst",
                                        name=f"kvst_{lid}_{th}")
                    for kk in range(KH):
                        k = KH * th + kk
                        ps = psB.tile([P, D], FP, tag="pb")
                        for i in range(k + 1):
                            nc.tensor.matmul(
                                ps[:], s_scT[th][:, i, kk * P:(kk + 1) * P],
                                s_xbf[i // KH][:, i % KH, :],
                                start=(i == 0), stop=(i == k))
                        nc.vector.tensor_copy(kvst[:, kk, :], ps[:])
                    nc.sync.dma_start(
                        ykv_in.rearrange("(o p) d -> p o d", p=P), kvst[:])
                    if comm:
                        nc.gpsimd.collective_compute(
                            "AllReduce", ADD,
                            replica_groups=[[0, 1], [2, 3], [4, 5], [6, 7]],
                            ins=[ykv_in.opt()], outs=[ykv_out.opt()])
                    else:
                        nc.sync.dma_start(ykv_out.opt(), ykv_in.opt())
                    ykv_out_h[th] = ykv_out

                def phase_D(th):
                    s_ykv = kv_pool.tile([P, KH, D], BF, tag="ykv",
                                         name=f"ykv_{lid}_{th}")
                    nc.sync.dma_start(
                        s_ykv[:],
                        ykv_out_h[th].rearrange("(o p) d -> p o d", p=P))
                    s_ykvn = kv_pool.tile([P, KH, D], BF, tag="ykvn",
                                          name=f"ykvn_{lid}_{th}")
                    ykvT = kv_pool.tile([P, DT, H], BF, tag="ykvT",
                                        name=f"ykvT_{lid}_{th}")
                    s_ykvT[th] = ykvT
                    mean, rstd = ln_batch(lambda kk: s_ykv[:, kk, :], KH)
                    for kk in range(KH):
                        nc.vector.tensor_scalar(
                            s_ykvn[:, kk, :], s_ykv[:, kk, :],
                            mean[:, kk:kk + 1], rstd[:, kk:kk + 1], SUB, MUL)
                        for dc in range(DT):
                            tp = psT.tile([P, P], BF, tag="trbf")
                            nc.tensor.transpose(
                                tp[:], s_ykvn[:, kk, dc * P:(dc + 1) * P],
                                ident_bf[:])
                            nc.scalar.copy(
                                ykvT[:, dc, kk * P:(kk + 1) * P], tp[:])

                def phase_Y(th):
                    t0 = th * H
                    encvb = {}
                    xr = None
                    for pi in range(FT):
                        if pi % 2 == 0:
                            xr = stage.tile([P, 4, H], BF, tag="stage",
                                            name=f"xr_{lid}_{th}_{pi}")
                            nc.sync.dma_start(
                                xr[:],
                                spill[pi * 2 * P:(pi + 2) * 2 * P, t0:t0 + H]
                                .rearrange("(o p) t -> p o t", p=P))
                        so = (pi % 2) * 2
                        for eo, idx in enumerate((pi, FT + pi)):
                            wq = fetch_w(d_encv, idx // 8, encvb,
                                         f"envv_{lid}_{th}")
                            col = (idx % 8) * P
                            xy_i = big.tile([P, H], BF, tag="big",
                                            name=f"xy_{lid}_{th}_{idx}")
                            ps = psA.tile([P, H], FP, tag="mm")
                            for dc in range(DT):
                                nc.tensor.matmul(
                                    ps[:], wq[:, dc, col:col + P],
                                    s_ykvT[th][:, dc, :],
                                    start=(dc == 0), stop=(dc == DT - 1))
                            yt = y_pool.tile([P, H], BF, tag="yt")
                            nc.scalar.activation(yt[:], ps[:], Relu)
                            nc.vector.tensor_tensor(
                                xy_i[:], xr[:, so + eo, :], yt[:], MUL)
                            xy[idx][th] = xy_i

                def phase_M(th):
                    ymlp_in = dram.tile([D, H], BF, tag="ymlp_in",
                                        name=f"ymi_{lid}_{th}")
                    ymlp_out = drsh.tile([D, H], BF, tag="ymlp_out",
                                         name=f"ymo_{lid}_{th}",
                                         addr_space="Shared")
                    decb = {}

                    def fetch_dec(q):
                        if q not in decb:
                            dq = w_pool.tile([P, 8, D], BF, tag="w",
                                             name=f"dec_{lid}_{th}_{q}")
                            nc.sync.dma_start(
                                dq[:], d_dec[:, q * 8:(q + 1) * 8, :])
                            decb[q] = dq
                        return decb[q]

                    pss = [psA.tile([P, H], FP, tag="mm",
                                    name=f"pss_{lid}_{th}_{dc}")
                           for dc in range(DT)]
                    for i in range(NT):
                        dq = fetch_dec(i // 8)
                        for dc in range(DT):
                            nc.tensor.matmul(
                                pss[dc][:], dq[:, i % 8, dc * P:(dc + 1) * P],
                                xy[i][th][:],
                                start=(i == 0), stop=(i == NT - 1))
                    ymst = ar_pool.tile([P, DT, H], BF, tag="arst",
                                        name=f"ymst_{lid}_{th}")
                    for dc in range(DT):
                        nc.vector.tensor_copy(ymst[:, dc, :], pss[dc][:])
                    nc.sync.dma_start(
                        ymlp_in.rearrange("(o p) t -> p o t", p=P), ymst[:])
                    if comm:
                        nc.gpsimd.collective_compute(
                            "AllReduce", ADD,
                            replica_groups=[[0, 1, 2, 3, 4, 5, 6, 7]],
                            ins=[ymlp_in.opt()], outs=[ymlp_out.opt()])
                    else:
                        nc.sync.dma_start(ymlp_out.opt(), ymlp_in.opt())
                    ymlp_out_h[th] = ymlp_out

                def phase_X(th):
                    s_ymT = kv_pool.tile([P, DT, H], BF, tag="ymT",
                                         name=f"ymT_{lid}_{th}")
                    nc.sync.dma_start(
                        s_ymT[:],
                        ymlp_out_h[th].rearrange("(o p) t -> p o t", p=P))
                    s_ym = kv_pool.tile([P, KH, D], FP, tag="ym",
                                        name=f"ym_{lid}_{th}")
                    for kk in range(KH):
                        for dc in range(DT):
                            tp = psT.tile([P, P], BF, tag="trbf")
                            nc.tensor.transpose(
                                tp[:], s_ymT[:, dc, kk * P:(kk + 1) * P],
                                ident_bf[:])
                            nc.vector.tensor_copy(
                                s_ym[:, kk, dc * P:(dc + 1) * P], tp[:])
                    mean1, rstd1 = ln_batch(lambda kk: s_ym[:, kk, :], KH)
                    us = []
                    for kk in range(KH):
                        u = u_pool.tile([P, D], FP, tag="u",
                                        name=f"u_{lid}_{th}_{kk}")
                        nc.vector.tensor_scalar(
                            u[:], s_ym[:, kk, :], mean1[:, kk:kk + 1],
                            rstd1[:, kk:kk + 1], SUB, MUL)
                        nc.vector.tensor_tensor(u[:], u[:], s_x[th][:, kk, :],
                                                ADD)
                        us.append(u)
                    mean2, rstd2 = ln_batch(lambda kk: us[kk][:], KH)
                    for kk in range(KH):
                        nc.vector.tensor_scalar(
                            s_x[th][:, kk, :], us[kk][:], mean2[:, kk:kk + 1],
                            rstd2[:, kk:kk + 1], SUB, MUL)
                        nc.vector.tensor_copy(s_xbf[th][:, kk, :],
                                              s_x[th][:, kk, :])
                    transpose_x_half(th)

                phase_A(0)
                if prev_x1 is not None:
                    prev_x1()      # prev layer's X(1): overlaps its AR behind A0
                phase_B(0)
                phase_A(1)
                phase_C(0)
                phase_B(1)
                phase_C(1)
                phase_D(0)
                phase_Y(0)
                phase_M(0)
                phase_D(1)
                phase_Y(1)
                phase_M(1)
                phase_X(0)
                return lambda: phase_X(1)

        prev = None
        for rep in range(reps):
            for layer in range(NL):
                prev = emit_layer(rep * NL + layer, prev)
        if prev is not None:
            prev()

        # ================= final head (fp32) =================
        s_xTf = big2.tile([P, DT, T], FP, tag="scT", name="xTf")
        for h in range(2):
            for kk in range(KH):
                for dc in range(DT):
                    tp = psB.tile([P, D], FP, tag="pb")
                    nc.tensor.transpose(
                        tp[:, :P], s_x[h][:, kk, dc * P:(dc + 1) * P],
                        ident_f[:])
                    nc.vector.tensor_copy(
                        s_xTf[:, dc, (h * KH + kk) * P:(h * KH + kk + 1) * P],
                        tp[:, :P])
        for k in range(KT):
            ps = psB.tile([P, VOCAB], FP, tag="pb")
            for dc in range(DT):
                nc.tensor.matmul(
                    ps[:], s_xTf[:, dc, k * P:(k + 1) * P], s_lmh[:, dc, :],
                    start=(dc == 0), stop=(dc == DT - 1))
            cp = cp_pool.tile([P, VOCAB], FP, tag="cpo", name=f"cpo_{k}")
            nc.vector.tensor_copy(cp[:], ps[:])
            nc.sync.dma_start(d_out[k * P:(k + 1) * P, :], cp[:])

    nc.compile()
    return nc


# --------------------------------------------------------------------------
# host side
# --------------------------------------------------------------------------

def _ln_np(x, eps=EPS):
    m = x.mean(-1, keepdims=True)
    v = ((x - m) ** 2).mean(-1, keepdims=True)
    return (x - m) / np.sqrt(v + eps)


def _prep_in_maps(inputs):
    idx = np.asarray(inputs["idx"]).reshape(-1).astype(np.int64)
    embed_w = np.asarray(inputs["embed_w"], dtype=np.float32)
    encoder = np.asarray(inputs["encoder"], dtype=np.float32)
    encoder_v = np.asarray(inputs["encoder_v"], dtype=np.float32)
    decoder = np.asarray(inputs["decoder"], dtype=np.float32).reshape(NH, N, D)
    lm_head = np.asarray(inputs["lm_head"], dtype=np.float32)

    x0 = _ln_np(embed_w[idx]).astype(np.float32)

    t_ = np.arange(N, dtype=np.float32)
    q = np.floor(t_ / np.float32(2.0)) * np.float32(2.0)
    freqs = (np.float32(1.0) / (np.float32(THETA) ** (q / np.float32(N)))
             / np.float32(TWO_PI))
    tt = np.arange(T, dtype=np.float32)
    phases = tt[:, None] * freqs[None, :]
    ph = np.mod(phases, np.float32(1.0)).astype(np.float32) * np.float32(TWO_PI)
    cos_full = np.cos(ph).astype(np.float32)
    sin_full = np.sin(ph).astype(np.float32)

    maskT = (np.arange(P)[:, None] < np.arange(P)[None, :]).astype(bf16)
    lmh = np.ascontiguousarray(
        lm_head.reshape(DT, P, VOCAB).transpose(1, 0, 2)).astype(np.float32)

    in_maps = []
    for c in range(8):
        h, half = c // 2, c % 2
        base = half * NSH
        perm = np.concatenate(
            [base + 2 * np.arange(F), base + 2 * np.arange(F) + 1])
        enc_c = encoder[h][:, perm]
        encv_c = encoder_v[h][:, perm]
        dec_c = decoder[h][perm, :]
        pcols = 2 * (half * F + np.arange(F))
        in_maps.append({
            "x0": x0,
            "cosT": np.ascontiguousarray(cos_full[:, pcols].T).astype(bf16),
            "sinT": np.ascontiguousarray(sin_full[:, pcols].T).astype(bf16),
            "enc": np.ascontiguousarray(
                enc_c.reshape(DT, P, NSH).transpose(1, 0, 2)).astype(bf16),
            "encv": np.ascontiguousarray(
                encv_c.reshape(DT, P, NSH).transpose(1, 0, 2)).astype(bf16),
            "dec": np.ascontiguousarray(
                dec_c.reshape(NT, P, D).transpose(1, 0, 2)).astype(bf16),
            "maskT": maskT,
            "lmh": lmh,
        })
    return in_maps


def kernel(**inputs) -> np.ndarray:
    in_maps = _prep_in_maps(inputs)
    if "nc" not in _CACHE:
        _CACHE["nc"] = _build_nc()
    res = run_bass_kernel_spmd(_CACHE["nc"], in_maps, core_ids=list(range(8)))
    logits = np.asarray(res.results[0]["logits"], dtype=np.float32)
    return logits.reshape(1, T, VOCAB)
